# revision 22
# baseline (speedup 1.0000x reference)
"""Balanced Feature Pyramid (Libra R-CNN BFP) on 8 Trainium2 NeuronCores.

Sharding: core k -> (batch n = k//2, channel-half h = k%2).  Every core
redundantly computes the full 256-channel gathered feature + non-local block
for its batch (cheap), but loads/keeps resident only its own 128-channel half
of the pyramid and writes only that half of the outputs.  To keep the program
identical across cores (SPMD), channels are permuted host-side so that the
"own" half is always channels 0..127 in program space; conv weights are
permuted accordingly (w' = P w P^T), which leaves the math exact.

The non-local block is computed without materializing the 1024x1024 attention
matrix: with augmented weights (bias folded as an extra input channel backed
by a ones-row in bsf~),
    z = Wo_lo @ g_x @ pw^T + bo
      = [Wo_lo Wg~] (bsf~ bsf~^T / N) [Wp~^T Wt~] bsf~ + bo
so everything flows through 257x257 intermediates (S~ = bsf~ bsf~^T/N,
T1 = S~ M2~, T2T = T1^T A~^T, z = T2T^T bsf~) - exact fp32 throughout.

Phases (single kernel launch, no collectives):
  A: load own half resident; stream+pool other half; assemble bsf.
  C: transposed bsf~ via PE transposes, then the chain above, group norm via
     0/1-mask matmuls + activation accumulators, residual.
  D: scatter: out_i = resize(bsf_refined)[own] + f_i[own], in place.
"""
import contextlib

import numpy as np

import concourse.bass as bass
import concourse.tile as tile
from concourse import bacc, mybir
from concourse.bass_utils import run_bass_kernel_spmd

F32 = mybir.dt.float32
AF = mybir.ActivationFunctionType
AX = mybir.AxisListType
ALU = mybir.AluOpType

N_CORES = 8
C = 256
H = 128  # channels per core (own half)
GS = 32
P = GS * GS  # 1024 positions
CA = C + 1  # augmented channels (ones row)
GROUPS_LO = 16
EPS = 1e-5

SIZES = [128, 64, 32, 16, 8]

_CACHE = {}


def build_program():
    nc = bacc.Bacc(
        "TRN2",
        target_bir_lowering=False,
        debug=False,
        enable_asserts=False,
        num_devices=N_CORES,
    )

    dram = {}
    for i, s in enumerate(SIZES):
        dram[f"f{i}o"] = nc.dram_tensor(f"f{i}o", [H, s * s], F32, kind="ExternalInput").ap()
        dram[f"f{i}x"] = nc.dram_tensor(f"f{i}x", [H, s * s], F32, kind="ExternalInput").ap()
    dram["wtA"] = nc.dram_tensor("wtA", [C, CA], F32, kind="ExternalInput").ap()
    dram["wpA"] = nc.dram_tensor("wpA", [C, CA], F32, kind="ExternalInput").ap()
    dram["wgA"] = nc.dram_tensor("wgA", [C, CA], F32, kind="ExternalInput").ap()
    dram["wol"] = nc.dram_tensor("wol", [C, H], F32, kind="ExternalInput").ap()
    dram["bo"] = nc.dram_tensor("bo", [C], F32, kind="ExternalInput").ap()
    dram["gg"] = nc.dram_tensor("gg", [H], F32, kind="ExternalInput").ap()
    dram["gb2"] = nc.dram_tensor("gb2", [H], F32, kind="ExternalInput").ap()
    dram["gmask"] = nc.dram_tensor("gmask", [128, GROUPS_LO], F32, kind="ExternalInput").ap()
    dram["bmask"] = nc.dram_tensor("bmask", [GROUPS_LO, 128], F32, kind="ExternalInput").ap()
    dram["ident"] = nc.dram_tensor("ident", [128, 128], F32, kind="ExternalInput").ap()
    for i, s in enumerate(SIZES):
        dram[f"o{i}"] = nc.dram_tensor(f"o{i}", [H, s * s], F32, kind="ExternalOutput").ap()

    with tile.TileContext(nc) as tc:
        _build_body(nc, tc, dram)

    nc.compile()
    return nc


def _build_body(nc, tc, dram):
    ctx = contextlib.ExitStack()
    with ctx:
        res = ctx.enter_context(tc.tile_pool(name="resident", bufs=1))
        stream = ctx.enter_context(tc.tile_pool(name="stream", bufs=1))
        vpool = ctx.enter_context(tc.tile_pool(name="vpool", bufs=3))
        consts = ctx.enter_context(tc.tile_pool(name="consts", bufs=1))
        work = ctx.enter_context(tc.tile_pool(name="work", bufs=1))
        out_stage = ctx.enter_context(tc.tile_pool(name="ostage", bufs=2))
        psum = ctx.enter_context(tc.tile_pool(name="psum", bufs=3, space="PSUM"))
        psum2 = ctx.enter_context(tc.tile_pool(name="psum2", bufs=2, space="PSUM"))
        psum_z = ctx.enter_context(tc.tile_pool(name="psum_z", bufs=1, space="PSUM"))

        # ======== phase A load issuance ========
        # small levels first: they gate the Bs accumulators that every
        # B-block finalize needs, so they must land before the f0 stream ends
        B0 = work.tile([128, P], F32, tag="B0")
        B1 = work.tile([128, P], F32, tag="B1")
        f2o = res.tile([128, P], F32, tag="f2o")
        nc.sync.dma_start(f2o[:], dram["f2o"][:])
        f2x = stream.tile([128, P], F32, tag="f2x")
        nc.sync.dma_start(f2x[:], dram["f2x"][:])
        f3o = res.tile([128, 256], F32, tag="f3o")
        nc.sync.dma_start(f3o[:], dram["f3o"][:])
        f3x = stream.tile([128, 256], F32, tag="f3x")
        nc.sync.dma_start(f3x[:], dram["f3x"][:])
        f4o = res.tile([128, 64], F32, tag="f4o")
        nc.sync.dma_start(f4o[:], dram["f4o"][:])
        f4x = stream.tile([128, 64], F32, tag="f4x")
        nc.sync.dma_start(f4x[:], dram["f4x"][:])
        f1o = res.tile([128, 4096], F32, tag="f1o")
        for b in range(2):
            nc.sync.dma_start(
                f1o[:, b * 2048 : (b + 1) * 2048], dram["f1o"][:, b * 2048 : (b + 1) * 2048]
            )
        v1x = stream.tile([128, 4096], F32, tag="v1x")
        for b in range(2):
            nc.sync.dma_start(
                v1x[:, b * 2048 : (b + 1) * 2048], dram["f1x"][:, b * 2048 : (b + 1) * 2048]
            )
        f0o = res.tile([128, 16384], F32, tag="f0o")
        for b in range(4):
            nc.sync.dma_start(
                f0o[:, b * 4096 : (b + 1) * 4096], dram["f0o"][:, b * 4096 : (b + 1) * 4096]
            )
        v0xs = []
        for b in range(8):
            v0x = vpool.tile([128, 2048], F32, tag="v0x", name=f"v0x{b}", bufs=6)
            nc.sync.dma_start(v0x[:], dram["f0x"][:, b * 2048 : (b + 1) * 2048])
            v0xs.append(v0x)

        # f1 pools first (DVE) - they gate Bs
        p1o = vpool.tile([128, P], F32, tag="scratch", name="p1o", bufs=2)
        v1o = f1o[:].rearrange("p (h dy w dx) -> p h w dy dx", h=32, dy=2, w=32, dx=2)
        nc.vector.reduce_max(p1o[:], v1o, axis=AX.XY)
        p1x = vpool.tile([128, P], F32, tag="scratch", name="p1x", bufs=2)
        v1xv = v1x[:].rearrange("p (h dy w dx) -> p h w dy dx", h=32, dy=2, w=32, dx=2)
        nc.vector.reduce_max(p1x[:], v1xv, axis=AX.XY)

        # ======== const loads ========
        wA = {}
        for wname in ("wtA", "wpA", "wgA"):
            for k in range(2):
                t = consts.tile([128, CA], F32, tag=f"{wname}{k}", name=f"{wname}{k}")
                nc.sync.dma_start(t[:], dram[wname][k * 128 : (k + 1) * 128, :])
                wA[f"{wname}{k}"] = t
        wol = []
        for k in range(2):
            t = consts.tile([128, H], F32, tag=f"wol{k}", name=f"wol{k}")
            nc.sync.dma_start(t[:], dram["wol"][k * 128 : (k + 1) * 128, :])
            wol.append(t)
        ident = consts.tile([128, 128], F32, tag="ident")
        nc.sync.dma_start(ident[:], dram["ident"][:])
        bo0 = consts.tile([128, 1], F32, tag="bo0")
        nc.sync.dma_start(bo0[:], dram["bo"].rearrange("(c o) -> c o", o=1)[0:128])
        gg_t = consts.tile([128, 1], F32, tag="gg")
        nc.sync.dma_start(gg_t[:], dram["gg"].rearrange("(c o) -> c o", o=1))
        gb2_t = consts.tile([128, 1], F32, tag="gb2")
        nc.sync.dma_start(gb2_t[:], dram["gb2"].rearrange("(c o) -> c o", o=1))
        gmask = consts.tile([128, GROUPS_LO], F32, tag="gmask")
        nc.sync.dma_start(gmask[:], dram["gmask"][:])
        bmask = consts.tile([GROUPS_LO, 128], F32, tag="bmask")
        nc.sync.dma_start(bmask[:], dram["bmask"][:])
        eps_t = consts.tile([GROUPS_LO, 1], F32, tag="eps_t")
        nc.gpsimd.memset(eps_t[:], EPS)
        ones_row = consts.tile([1, P], F32, tag="ones_row")
        nc.gpsimd.memset(ones_row[:], 1.0)

        # ---- M2~ = WpA^T @ WtA  [257, 257] in chunks (128,128,1) ----
        M2 = []
        for m in range(2):
            pm = psum.tile([128, CA], F32, tag="mm", name=f"pm2_{m}", padded_shape=[128, 512])
            for k in range(2):
                nc.tensor.matmul(
                    pm[:],
                    wA[f"wpA{k}"][:, m * 128 : (m + 1) * 128],
                    wA[f"wtA{k}"][:],
                    start=(k == 0),
                    stop=(k == 1),
                )
            t = consts.tile([128, CA], F32, tag=f"M2_{m}", name=f"M2_{m}")
            nc.scalar.copy(t[:], pm[:])
            M2.append(t)
        pmr = psum.tile([1, CA], F32, tag="mm", name="pm2r", padded_shape=[128, 512])
        for k in range(2):
            nc.tensor.matmul(
                pmr[:], wA[f"wpA{k}"][:, 256:257], wA[f"wtA{k}"][:],
                start=(k == 0), stop=(k == 1),
            )
        M2r = consts.tile([1, CA], F32, tag="M2r")
        nc.scalar.copy(M2r[:], pmr[:])
        M2.append(M2r)

        # ---- AT~ = (Wo_lo @ WgA)^T  [257, 128] in chunks ----
        AT = []
        for m in range(2):
            pa = psum.tile([128, H], F32, tag="mm", name=f"pat{m}", padded_shape=[128, 512])
            for k in range(2):
                nc.tensor.matmul(
                    pa[:],
                    wA[f"wgA{k}"][:, m * 128 : (m + 1) * 128],
                    wol[k][:],
                    start=(k == 0),
                    stop=(k == 1),
                )
            t = consts.tile([128, H], F32, tag=f"AT{m}", name=f"AT{m}")
            nc.scalar.copy(t[:], pa[:])
            AT.append(t)
        par = psum.tile([1, H], F32, tag="mm", name="patr", padded_shape=[128, 512])
        for k in range(2):
            nc.tensor.matmul(
                par[:], wA[f"wgA{k}"][:, 256:257], wol[k][:],
                start=(k == 0), stop=(k == 1),
            )
        ATr = consts.tile([1, H], F32, tag="ATr")
        nc.scalar.copy(ATr[:], par[:])
        AT.append(ATr)

        # ======== phase A: bsf assembly ========
        def ups_add(B, small, hs, f, engine):
            for r in range(f):
                ov = B.rearrange("p (h r w s) -> p h r w s", h=hs, r=f, w=hs, s=f)[:, :, r]
                sv = small.rearrange("p (h w s) -> p h w s", h=hs, w=hs, s=1).broadcast_to(
                    [128, hs, hs, f]
                )
                engine.tensor_add(ov, ov, sv)

        # small-level accumulators built early (only small loads + f1 pools needed)
        Bs0 = work.tile([128, P], F32, tag="big4", name="Bs0", bufs=2)
        Bs1 = work.tile([128, P], F32, tag="big4", name="Bs1", bufs=2)
        nc.vector.tensor_add(Bs0[:], p1o[:], f2o[:])
        nc.vector.tensor_add(Bs1[:], p1x[:], f2x[:])
        ups_add(Bs0[:], f3o[:], 16, 2, nc.vector)
        ups_add(Bs1[:], f3x[:], 16, 2, nc.vector)
        ups_add(Bs0[:], f4o[:], 8, 4, nc.vector)
        ups_add(Bs1[:], f4x[:], 8, 4, nc.vector)
        # pre-scale the small-level accumulators (early, off the critical path)
        nc.scalar.mul(Bs0[:], Bs0[:], 0.2)
        nc.scalar.mul(Bs1[:], Bs1[:], 0.2)
        # f0 own pooling + B0 finalize per chunk
        for b in range(4):
            view = f0o[:, b * 4096 : (b + 1) * 4096].rearrange(
                "p (h dy w dx) -> p h w dy dx", h=8, dy=4, w=32, dx=4
            )
            nc.vector.reduce_max(B0[:, b * 256 : (b + 1) * 256], view, axis=AX.XY)
            sl = slice(b * 256, (b + 1) * 256)
            nc.vector.scalar_tensor_tensor(
                B0[:, sl], B0[:, sl], 0.2, Bs0[:, sl], op0=ALU.mult, op1=ALU.add
            )
        B = [B0, B1]

        # interleaved stream: pool f0x chunk q -> finalize B1 block q ->
        # transpose block q -> S~ accumulation step q
        bsfT = []
        pS = []
        for m in range(3):
            rows = 128 if m < 2 else 1
            pS.append(
                psum.tile([rows, CA], F32, tag="mm", name=f"pS{m}", padded_shape=[128, 512])
            )
        for q in range(8):
            sl = slice(q * 128, (q + 1) * 128)
            view = v0xs[q][:].rearrange(
                "p (h dy w dx) -> p h w dy dx", h=4, dy=4, w=32, dx=4
            )
            nc.vector.reduce_max(B1[:, sl], view, axis=AX.XY)
            nc.vector.scalar_tensor_tensor(
                B1[:, sl], B1[:, sl], 0.2, Bs1[:, sl], op0=ALU.mult, op1=ALU.add
            )
            pt = psum2.tile([128, 256], F32, tag="ptr", name=f"ptr{q}")
            for m in range(2):
                nc.tensor.transpose(
                    pt[:, m * 128 : (m + 1) * 128],
                    B[m][:, q * 128 : (q + 1) * 128],
                    ident[:],
                )
            t = work.tile([128, CA], F32, tag="bsfT", name=f"bsfT{q}", bufs=3)
            nc.scalar.copy(t[:, 0:256], pt[:])
            nc.gpsimd.memset(t[:, 256:257], 1.0)
            bsfT.append(t)
            for m in range(3):
                lhs = (
                    t[:, m * 128 : (m + 1) * 128] if m < 2 else t[:, 256:257]
                )
                nc.tensor.matmul(pS[m][:], lhs, t[:], start=(q == 0), stop=(q == 7))

        S = []
        for m in range(3):
            rows = 128 if m < 2 else 1
            t = work.tile([rows, CA], F32, tag=f"S{m}", name=f"S{m}")
            nc.scalar.mul(t[:], pS[m][:], 1.0 / P)
            S.append(t)

        # T1 = S~ @ M2~ (S symmetric -> S rows used as lhsT)
        T1 = []
        for m in range(3):
            rows = 128 if m < 2 else 1
            pT = psum.tile([rows, CA], F32, tag="mm", name=f"pT1{m}", padded_shape=[128, 512])
            for k in range(3):
                lhs = S[k][:, m * 128 : (m + 1) * 128] if m < 2 else S[k][:, 256:257]
                nc.tensor.matmul(pT[:], lhs, M2[k][:], start=(k == 0), stop=(k == 2))
            t = work.tile([rows, CA], F32, tag=f"T1_{m}", name=f"T1_{m}")
            nc.scalar.copy(t[:], pT[:])
            T1.append(t)

        # T2T = T1^T @ A~^T : [257, 128] chunks
        T2T = []
        for m in range(3):
            rows = 128 if m < 2 else 1
            pZ = psum.tile([rows, H], F32, tag="mm", name=f"pT2{m}", padded_shape=[128, 512])
            for k in range(3):
                lhs = T1[k][:, m * 128 : (m + 1) * 128] if m < 2 else T1[k][:, 256:257]
                nc.tensor.matmul(pZ[:], lhs, AT[k][:], start=(k == 0), stop=(k == 2))
            t = work.tile([rows, H], F32, tag=f"T2T{m}", name=f"T2T{m}")
            nc.scalar.copy(t[:], pZ[:])
            T2T.append(t)

        # z = T2T^T @ bsf~  [128, 1024]
        pz = psum_z.tile([128, P], F32, tag="pz")
        for i in range(2):
            sl = slice(i * 512, (i + 1) * 512)
            for k in range(3):
                rhs = B[k][:, sl] if k < 2 else ones_row[:, sl]
                nc.tensor.matmul(pz[:, sl], T2T[k][:], rhs, start=(k == 0), stop=(k == 2))

        # GN stats via activation accumulators + mask matmuls
        Z = work.tile([128, P], F32, tag="big4", name="Z", bufs=2)
        zsum = work.tile([128, 1], F32, tag="zsum")
        nc.scalar.activation(Z[:], pz[:], AF.Identity, bias=bo0[:], accum_out=zsum[:])
        Z2 = vpool.tile([128, P], F32, tag="scratch", name="Z2", bufs=2)
        z2sum = work.tile([128, 1], F32, tag="z2sum")
        nc.scalar.activation(Z2[:], pz[:], AF.Square, bias=bo0[:], accum_out=z2sum[:])

        ps = psum.tile([GROUPS_LO, 2], F32, tag="mm", name="ps", padded_shape=[128, 512])
        nc.tensor.matmul(ps[:, 0:1], gmask[:], zsum[:], start=True, stop=True)
        nc.tensor.matmul(ps[:, 1:2], gmask[:], z2sum[:], start=True, stop=True)
        stats = work.tile([GROUPS_LO, 4], F32, tag="stats")
        mu = stats[:, 0:1]
        e2 = stats[:, 1:2]
        nc.scalar.mul(mu, ps[:, 0:1], 1.0 / (8 * P))
        nc.scalar.mul(e2, ps[:, 1:2], 1.0 / (8 * P))
        var = stats[:, 2:3]
        nc.vector.tensor_mul(var, mu, mu)
        nc.vector.tensor_sub(var, e2, var)
        sd = stats[:, 3:4]
        nc.scalar.activation(sd, var, AF.Sqrt, bias=eps_t[:])
        muinv = work.tile([GROUPS_LO, 2], F32, tag="muinv")
        nc.vector.tensor_copy(muinv[:, 0:1], mu)
        nc.vector.reciprocal(muinv[:, 1:2], sd)
        pbc = psum.tile([128, 2], F32, tag="mm", name="pbc", padded_shape=[128, 512])
        nc.tensor.matmul(pbc[:], bmask[:], muinv[:], start=True, stop=True)
        chan = work.tile([128, 4], F32, tag="chan")
        nc.vector.tensor_copy(chan[:, 0:2], pbc[:])
        mu_c = chan[:, 0:1]
        inv_c = chan[:, 1:2]
        s_c = chan[:, 2:3]
        t_c = chan[:, 3:4]
        nc.vector.tensor_mul(s_c, gg_t[:], inv_c)
        nc.vector.tensor_mul(t_c, mu_c, s_c)
        nc.vector.tensor_sub(t_c, gb2_t[:], t_c)

        R = work.tile([128, P], F32, tag="big4", name="R", bufs=2)
        nc.vector.tensor_scalar(R[:], Z[:], s_c, t_c, op0=ALU.mult, op1=ALU.add)
        nc.vector.tensor_add(R[:], R[:], B0[:])

        # ======== phase D: scatter (in-place into resident tiles) ========
        nc.vector.tensor_add(f2o[:], R[:], f2o[:])
        nc.sync.dma_start(dram["o2"][:], f2o[:])

        o3t = out_stage.tile([128, 256], F32, tag="o3t", bufs=1)
        nc.vector.reduce_max(
            o3t[:],
            R[:].rearrange("p (h dy w dx) -> p h w dy dx", h=16, dy=2, w=16, dx=2),
            axis=AX.XY,
        )
        nc.vector.tensor_add(o3t[:], o3t[:], f3o[:])
        nc.sync.dma_start(dram["o3"][:], o3t[:])

        o4t = out_stage.tile([128, 64], F32, tag="o4t", bufs=1)
        nc.vector.reduce_max(
            o4t[:],
            R[:].rearrange("p (h dy w dx) -> p h w dy dx", h=8, dy=4, w=8, dx=4),
            axis=AX.XY,
        )
        nc.vector.tensor_add(o4t[:], o4t[:], f4o[:])
        nc.sync.dma_start(dram["o4"][:], o4t[:])

        for b in range(2):
            ov = f1o[:, b * 2048 : (b + 1) * 2048].rearrange(
                "p (h r w s) -> p h r w s", h=16, r=2, w=32, s=2
            )
            sv = (
                R[:, b * 512 : (b + 1) * 512]
                .rearrange("p (h w s) -> p h w s", h=16, w=32, s=1)
                .broadcast_to([128, 16, 32, 2])
            )
            for r in range(2):
                nc.vector.tensor_add(ov[:, :, r], ov[:, :, r], sv)
            nc.sync.dma_start(
                dram["o1"][:, b * 2048 : (b + 1) * 2048], f1o[:, b * 2048 : (b + 1) * 2048]
            )
        for b in range(4):
            ov = f0o[:, b * 4096 : (b + 1) * 4096].rearrange(
                "p (h r w s) -> p h r w s", h=8, r=4, w=32, s=4
            )
            sv = (
                R[:, b * 256 : (b + 1) * 256]
                .rearrange("p (h w s) -> p h w s", h=8, w=32, s=1)
                .broadcast_to([128, 8, 32, 4])
            )
            for r in range(4):
                nc.vector.tensor_add(ov[:, :, r], ov[:, :, r], sv)
            nc.sync.dma_start(
                dram["o0"][:, b * 4096 : (b + 1) * 4096], f0o[:, b * 4096 : (b + 1) * 4096]
            )


def _get_program():
    if "nc" not in _CACHE:
        _CACHE["nc"] = build_program()
    return _CACHE["nc"]


def kernel(**inputs):
    f = [np.asarray(inputs[f"f{i}"], np.float32) for i in range(5)]
    n_batch = f[0].shape[0]
    w = {k: np.asarray(inputs[k], np.float32) for k in ("g_w", "th_w", "ph_w", "out_w")}
    b = {k: np.asarray(inputs[k], np.float32) for k in ("g_b", "th_b", "ph_b", "out_b")}
    gn_g = np.asarray(inputs["gn_g"], np.float32)
    gn_b = np.asarray(inputs["gn_b"], np.float32)

    perms = [np.arange(C), np.concatenate([np.arange(128, 256), np.arange(0, 128)])]

    gm = np.zeros((128, GROUPS_LO), np.float32)
    gm[np.arange(128), np.arange(128) // 8] = 1.0
    ident = np.eye(128, dtype=np.float32)

    in_maps = []
    for k in range(N_CORES):
        n, h = k // 2, k % 2
        pm = perms[h]
        m = {}
        for i in range(5):
            fi = f[i][n][pm]
            s = SIZES[i]
            m[f"f{i}o"] = np.ascontiguousarray(fi[:H].reshape(H, s * s))
            m[f"f{i}x"] = np.ascontiguousarray(fi[H:].reshape(H, s * s))
        # permuted, bias-augmented weights
        for name, wk, bk in (
            ("wtA", "th_w", "th_b"),
            ("wpA", "ph_w", "ph_b"),
            ("wgA", "g_w", "g_b"),
        ):
            wp = w[wk][pm][:, pm]
            bp = b[bk][pm]
            m[name] = np.ascontiguousarray(np.concatenate([wp, bp[:, None]], axis=1))
        wo_p = w["out_w"][pm][:, pm]
        m["wol"] = np.ascontiguousarray(wo_p[:H, :].T)
        m["bo"] = np.ascontiguousarray(b["out_b"][pm])
        m["gg"] = np.ascontiguousarray(gn_g[pm][:H])
        m["gb2"] = np.ascontiguousarray(gn_b[pm][:H])
        m["gmask"] = gm
        m["bmask"] = np.ascontiguousarray(gm.T)
        m["ident"] = ident
        in_maps.append(m)

    nc = _get_program()
    kw = {}
    if _CACHE.get("profile"):
        kw["trace"] = True
    res = run_bass_kernel_spmd(nc, in_maps, core_ids=list(range(N_CORES)), **kw)
    _CACHE["last_result"] = res

    outs = []
    for i, s in enumerate(SIZES):
        o = np.empty((n_batch, C, s, s), np.float32)
        for k in range(N_CORES):
            n, h = k // 2, k % 2
            pm = perms[h]
            o[n, pm[:H]] = res.results[k][f"o{i}"].reshape(H, s, s)
        outs.append(o)
    return tuple(outs)


# revision 23
# speedup vs baseline: 1.1181x; 1.1181x over previous
"""Balanced Feature Pyramid (Libra R-CNN BFP) on 8 Trainium2 NeuronCores.

Sharding: core k -> (batch n = k//2, channel-half h = k%2).  Every core
redundantly computes the full 256-channel gathered feature + non-local block
for its batch (cheap), but loads/keeps resident only its own 128-channel half
of the pyramid and writes only that half of the outputs.  To keep the program
identical across cores (SPMD), channels are permuted host-side so that the
"own" half is always channels 0..127 in program space; conv weights are
permuted accordingly (w' = P w P^T), which leaves the math exact.

The non-local block is computed without materializing the 1024x1024 attention
matrix: with augmented weights (bias folded as an extra input channel backed
by a ones-row in bsf~),
    z = Wo_lo @ g_x @ pw^T + bo
      = [Wo_lo Wg~] (bsf~ bsf~^T / N) [Wp~^T Wt~] bsf~ + bo
so everything flows through 257x257 intermediates (S~ = bsf~ bsf~^T/N,
T1 = S~ M2~, T2T = T1^T A~^T, z = T2T^T bsf~) - exact fp32 throughout.

Phases (single kernel launch, no collectives):
  A: load own half resident; stream+pool other half; assemble bsf.
  C: transposed bsf~ via PE transposes, then the chain above, group norm via
     0/1-mask matmuls + activation accumulators, residual.
  D: scatter: out_i = resize(bsf_refined)[own] + f_i[own], in place.
"""
import contextlib

import numpy as np

import concourse.bass as bass
import concourse.tile as tile
from concourse import bacc, mybir
from concourse.bass_utils import run_bass_kernel_spmd

F32 = mybir.dt.float32
AF = mybir.ActivationFunctionType
AX = mybir.AxisListType
ALU = mybir.AluOpType

N_CORES = 8
C = 256
H = 128  # channels per core (own half)
GS = 32
P = GS * GS  # 1024 positions
CA = C + 1  # augmented channels (ones row)
GROUPS_LO = 16
EPS = 1e-5

SIZES = [128, 64, 32, 16, 8]

_CACHE = {}


def build_program():
    nc = bacc.Bacc(
        "TRN2",
        target_bir_lowering=False,
        debug=False,
        enable_asserts=False,
        num_devices=N_CORES,
    )

    dram = {}
    for i, s in enumerate(SIZES):
        dram[f"f{i}o"] = nc.dram_tensor(f"f{i}o", [H, s * s], F32, kind="ExternalInput").ap()
        dram[f"f{i}x"] = nc.dram_tensor(f"f{i}x", [H, s * s], F32, kind="ExternalInput").ap()
    dram["wtA"] = nc.dram_tensor("wtA", [C, CA], F32, kind="ExternalInput").ap()
    dram["wpA"] = nc.dram_tensor("wpA", [C, CA], F32, kind="ExternalInput").ap()
    dram["wgA"] = nc.dram_tensor("wgA", [C, CA], F32, kind="ExternalInput").ap()
    dram["wol"] = nc.dram_tensor("wol", [C, H], F32, kind="ExternalInput").ap()
    dram["bo"] = nc.dram_tensor("bo", [C], F32, kind="ExternalInput").ap()
    dram["gg"] = nc.dram_tensor("gg", [H], F32, kind="ExternalInput").ap()
    dram["gb2"] = nc.dram_tensor("gb2", [H], F32, kind="ExternalInput").ap()
    dram["gmask"] = nc.dram_tensor("gmask", [128, GROUPS_LO], F32, kind="ExternalInput").ap()
    dram["bmask"] = nc.dram_tensor("bmask", [GROUPS_LO, 128], F32, kind="ExternalInput").ap()
    dram["ident"] = nc.dram_tensor("ident", [128, 128], F32, kind="ExternalInput").ap()
    for i, s in enumerate(SIZES):
        dram[f"o{i}"] = nc.dram_tensor(f"o{i}", [H, s * s], F32, kind="ExternalOutput").ap()

    with tile.TileContext(nc) as tc:
        _build_body(nc, tc, dram)

    nc.compile()
    return nc


def _build_body(nc, tc, dram):
    ctx = contextlib.ExitStack()
    with ctx:
        res = ctx.enter_context(tc.tile_pool(name="resident", bufs=1))
        stream = ctx.enter_context(tc.tile_pool(name="stream", bufs=1))
        vpool = ctx.enter_context(tc.tile_pool(name="vpool", bufs=3))
        consts = ctx.enter_context(tc.tile_pool(name="consts", bufs=1))
        work = ctx.enter_context(tc.tile_pool(name="work", bufs=1))
        out_stage = ctx.enter_context(tc.tile_pool(name="ostage", bufs=2))
        psum = ctx.enter_context(tc.tile_pool(name="psum", bufs=3, space="PSUM"))
        psum2 = ctx.enter_context(tc.tile_pool(name="psum2", bufs=2, space="PSUM"))
        psum_z = ctx.enter_context(tc.tile_pool(name="psum_z", bufs=1, space="PSUM"))

        # ======== phase A load issuance ========
        B0 = work.tile([128, P], F32, tag="B0")
        B1 = work.tile([128, P], F32, tag="B1")
        f1o = res.tile([128, 4096], F32, tag="f1o")
        for b in range(2):
            nc.sync.dma_start(
                f1o[:, b * 2048 : (b + 1) * 2048], dram["f1o"][:, b * 2048 : (b + 1) * 2048]
            )
        v1x = stream.tile([128, 4096], F32, tag="v1x")
        for b in range(2):
            nc.sync.dma_start(
                v1x[:, b * 2048 : (b + 1) * 2048], dram["f1x"][:, b * 2048 : (b + 1) * 2048]
            )
        f2o = res.tile([128, P], F32, tag="f2o")
        nc.sync.dma_start(f2o[:], dram["f2o"][:])
        f2x = stream.tile([128, P], F32, tag="f2x")
        nc.sync.dma_start(f2x[:], dram["f2x"][:])
        f3o = res.tile([128, 256], F32, tag="f3o")
        nc.sync.dma_start(f3o[:], dram["f3o"][:])
        f3x = stream.tile([128, 256], F32, tag="f3x")
        nc.sync.dma_start(f3x[:], dram["f3x"][:])
        f4o = res.tile([128, 64], F32, tag="f4o")
        nc.sync.dma_start(f4o[:], dram["f4o"][:])
        f4x = stream.tile([128, 64], F32, tag="f4x")
        nc.sync.dma_start(f4x[:], dram["f4x"][:])
        f0o = res.tile([128, 16384], F32, tag="f0o")
        for b in range(4):
            nc.sync.dma_start(
                f0o[:, b * 4096 : (b + 1) * 4096], dram["f0o"][:, b * 4096 : (b + 1) * 4096]
            )
        v0xs = []
        for b in range(8):
            v0x = vpool.tile([128, 2048], F32, tag="v0x", name=f"v0x{b}", bufs=6)
            nc.sync.dma_start(v0x[:], dram["f0x"][:, b * 2048 : (b + 1) * 2048])
            v0xs.append(v0x)

        # f1 pools first (DVE) - they gate Bs
        p1o = vpool.tile([128, P], F32, tag="scratch", name="p1o", bufs=2)
        v1o = f1o[:].rearrange("p (h dy w dx) -> p h w dy dx", h=32, dy=2, w=32, dx=2)
        nc.vector.reduce_max(p1o[:], v1o, axis=AX.XY)
        p1x = vpool.tile([128, P], F32, tag="scratch", name="p1x", bufs=2)
        v1xv = v1x[:].rearrange("p (h dy w dx) -> p h w dy dx", h=32, dy=2, w=32, dx=2)
        nc.vector.reduce_max(p1x[:], v1xv, axis=AX.XY)

        # ======== const loads ========
        wA = {}
        for wname in ("wtA", "wpA", "wgA"):
            for k in range(2):
                t = consts.tile([128, CA], F32, tag=f"{wname}{k}", name=f"{wname}{k}")
                nc.sync.dma_start(t[:], dram[wname][k * 128 : (k + 1) * 128, :])
                wA[f"{wname}{k}"] = t
        wol = []
        for k in range(2):
            t = consts.tile([128, H], F32, tag=f"wol{k}", name=f"wol{k}")
            nc.sync.dma_start(t[:], dram["wol"][k * 128 : (k + 1) * 128, :])
            wol.append(t)
        ident = consts.tile([128, 128], F32, tag="ident")
        nc.sync.dma_start(ident[:], dram["ident"][:])
        bo0 = consts.tile([128, 1], F32, tag="bo0")
        nc.sync.dma_start(bo0[:], dram["bo"].rearrange("(c o) -> c o", o=1)[0:128])
        gg_t = consts.tile([128, 1], F32, tag="gg")
        nc.sync.dma_start(gg_t[:], dram["gg"].rearrange("(c o) -> c o", o=1))
        gb2_t = consts.tile([128, 1], F32, tag="gb2")
        nc.sync.dma_start(gb2_t[:], dram["gb2"].rearrange("(c o) -> c o", o=1))
        gmask = consts.tile([128, GROUPS_LO], F32, tag="gmask")
        nc.sync.dma_start(gmask[:], dram["gmask"][:])
        bmask = consts.tile([GROUPS_LO, 128], F32, tag="bmask")
        nc.sync.dma_start(bmask[:], dram["bmask"][:])
        eps_t = consts.tile([GROUPS_LO, 1], F32, tag="eps_t")
        nc.gpsimd.memset(eps_t[:], EPS)
        ones_row = consts.tile([1, P], F32, tag="ones_row")
        nc.gpsimd.memset(ones_row[:], 1.0)

        # ---- M2~ = WpA^T @ WtA  [257, 257] in chunks (128,128,1) ----
        M2 = []
        for m in range(2):
            pm = psum.tile([128, CA], F32, tag="mm", name=f"pm2_{m}", padded_shape=[128, 512])
            for k in range(2):
                nc.tensor.matmul(
                    pm[:],
                    wA[f"wpA{k}"][:, m * 128 : (m + 1) * 128],
                    wA[f"wtA{k}"][:],
                    start=(k == 0),
                    stop=(k == 1),
                )
            t = consts.tile([128, CA], F32, tag=f"M2_{m}", name=f"M2_{m}")
            nc.scalar.copy(t[:], pm[:])
            M2.append(t)
        pmr = psum.tile([1, CA], F32, tag="mm", name="pm2r", padded_shape=[128, 512])
        for k in range(2):
            nc.tensor.matmul(
                pmr[:], wA[f"wpA{k}"][:, 256:257], wA[f"wtA{k}"][:],
                start=(k == 0), stop=(k == 1),
            )
        M2r = consts.tile([1, CA], F32, tag="M2r")
        nc.scalar.copy(M2r[:], pmr[:])
        M2.append(M2r)

        # ---- AT~ = (Wo_lo @ WgA)^T  [257, 128] in chunks ----
        AT = []
        for m in range(2):
            pa = psum.tile([128, H], F32, tag="mm", name=f"pat{m}", padded_shape=[128, 512])
            for k in range(2):
                nc.tensor.matmul(
                    pa[:],
                    wA[f"wgA{k}"][:, m * 128 : (m + 1) * 128],
                    wol[k][:],
                    start=(k == 0),
                    stop=(k == 1),
                )
            t = consts.tile([128, H], F32, tag=f"AT{m}", name=f"AT{m}")
            nc.scalar.copy(t[:], pa[:])
            AT.append(t)
        par = psum.tile([1, H], F32, tag="mm", name="patr", padded_shape=[128, 512])
        for k in range(2):
            nc.tensor.matmul(
                par[:], wA[f"wgA{k}"][:, 256:257], wol[k][:],
                start=(k == 0), stop=(k == 1),
            )
        ATr = consts.tile([1, H], F32, tag="ATr")
        nc.scalar.copy(ATr[:], par[:])
        AT.append(ATr)

        # ======== phase A: bsf assembly ========
        def ups_add(B, small, hs, f, engine):
            for r in range(f):
                ov = B.rearrange("p (h r w s) -> p h r w s", h=hs, r=f, w=hs, s=f)[:, :, r]
                sv = small.rearrange("p (h w s) -> p h w s", h=hs, w=hs, s=1).broadcast_to(
                    [128, hs, hs, f]
                )
                engine.tensor_add(ov, ov, sv)

        # small-level accumulators built early (only small loads + f1 pools needed)
        Bs0 = work.tile([128, P], F32, tag="big4", name="Bs0", bufs=2)
        Bs1 = work.tile([128, P], F32, tag="big4", name="Bs1", bufs=2)
        nc.vector.tensor_add(Bs0[:], p1o[:], f2o[:])
        nc.vector.tensor_add(Bs1[:], p1x[:], f2x[:])
        ups_add(Bs0[:], f3o[:], 16, 2, nc.vector)
        ups_add(Bs1[:], f3x[:], 16, 2, nc.vector)
        ups_add(Bs0[:], f4o[:], 8, 4, nc.vector)
        ups_add(Bs1[:], f4x[:], 8, 4, nc.vector)
        # pre-scale the small-level accumulators (early, off the critical path)
        nc.scalar.mul(Bs0[:], Bs0[:], 0.2)
        nc.scalar.mul(Bs1[:], Bs1[:], 0.2)
        # f0 own pooling + B0 finalize per chunk
        for b in range(4):
            view = f0o[:, b * 4096 : (b + 1) * 4096].rearrange(
                "p (h dy w dx) -> p h w dy dx", h=8, dy=4, w=32, dx=4
            )
            nc.vector.reduce_max(B0[:, b * 256 : (b + 1) * 256], view, axis=AX.XY)
            sl = slice(b * 256, (b + 1) * 256)
            nc.vector.scalar_tensor_tensor(
                B0[:, sl], B0[:, sl], 0.2, Bs0[:, sl], op0=ALU.mult, op1=ALU.add
            )
        B = [B0, B1]

        # interleaved stream: pool f0x chunk q -> finalize B1 block q ->
        # transpose block q -> S~ accumulation step q
        bsfT = []
        pS = []
        for m in range(3):
            rows = 128 if m < 2 else 1
            pS.append(
                psum.tile([rows, CA], F32, tag="mm", name=f"pS{m}", padded_shape=[128, 512])
            )
        for q in range(8):
            sl = slice(q * 128, (q + 1) * 128)
            view = v0xs[q][:].rearrange(
                "p (h dy w dx) -> p h w dy dx", h=4, dy=4, w=32, dx=4
            )
            nc.vector.reduce_max(B1[:, sl], view, axis=AX.XY)
            nc.vector.scalar_tensor_tensor(
                B1[:, sl], B1[:, sl], 0.2, Bs1[:, sl], op0=ALU.mult, op1=ALU.add
            )
            pt = psum2.tile([128, 256], F32, tag="ptr", name=f"ptr{q}")
            for m in range(2):
                nc.tensor.transpose(
                    pt[:, m * 128 : (m + 1) * 128],
                    B[m][:, q * 128 : (q + 1) * 128],
                    ident[:],
                )
            t = work.tile([128, CA], F32, tag="bsfT", name=f"bsfT{q}", bufs=3)
            nc.scalar.copy(t[:, 0:256], pt[:])
            nc.gpsimd.memset(t[:, 256:257], 1.0)
            bsfT.append(t)
            for m in range(3):
                lhs = (
                    t[:, m * 128 : (m + 1) * 128] if m < 2 else t[:, 256:257]
                )
                nc.tensor.matmul(pS[m][:], lhs, t[:], start=(q == 0), stop=(q == 7))

        S = []
        for m in range(3):
            rows = 128 if m < 2 else 1
            t = work.tile([rows, CA], F32, tag=f"S{m}", name=f"S{m}")
            nc.scalar.mul(t[:], pS[m][:], 1.0 / P)
            S.append(t)

        # T1 = S~ @ M2~ (S symmetric -> S rows used as lhsT)
        T1 = []
        for m in range(3):
            rows = 128 if m < 2 else 1
            pT = psum.tile([rows, CA], F32, tag="mm", name=f"pT1{m}", padded_shape=[128, 512])
            for k in range(3):
                lhs = S[k][:, m * 128 : (m + 1) * 128] if m < 2 else S[k][:, 256:257]
                nc.tensor.matmul(pT[:], lhs, M2[k][:], start=(k == 0), stop=(k == 2))
            t = work.tile([rows, CA], F32, tag=f"T1_{m}", name=f"T1_{m}")
            nc.scalar.copy(t[:], pT[:])
            T1.append(t)

        # T2T = T1^T @ A~^T : [257, 128] chunks
        T2T = []
        for m in range(3):
            rows = 128 if m < 2 else 1
            pZ = psum.tile([rows, H], F32, tag="mm", name=f"pT2{m}", padded_shape=[128, 512])
            for k in range(3):
                lhs = T1[k][:, m * 128 : (m + 1) * 128] if m < 2 else T1[k][:, 256:257]
                nc.tensor.matmul(pZ[:], lhs, AT[k][:], start=(k == 0), stop=(k == 2))
            t = work.tile([rows, H], F32, tag=f"T2T{m}", name=f"T2T{m}")
            nc.scalar.copy(t[:], pZ[:])
            T2T.append(t)

        # z = T2T^T @ bsf~  [128, 1024]
        pz = psum_z.tile([128, P], F32, tag="pz")
        for i in range(2):
            sl = slice(i * 512, (i + 1) * 512)
            for k in range(3):
                rhs = B[k][:, sl] if k < 2 else ones_row[:, sl]
                nc.tensor.matmul(pz[:, sl], T2T[k][:], rhs, start=(k == 0), stop=(k == 2))

        # GN stats via activation accumulators + mask matmuls
        Z = work.tile([128, P], F32, tag="big4", name="Z", bufs=2)
        zsum = work.tile([128, 1], F32, tag="zsum")
        nc.scalar.activation(Z[:], pz[:], AF.Identity, bias=bo0[:], accum_out=zsum[:])
        Z2 = vpool.tile([128, P], F32, tag="scratch", name="Z2", bufs=2)
        z2sum = work.tile([128, 1], F32, tag="z2sum")
        nc.scalar.activation(Z2[:], pz[:], AF.Square, bias=bo0[:], accum_out=z2sum[:])

        ps = psum.tile([GROUPS_LO, 2], F32, tag="mm", name="ps", padded_shape=[128, 512])
        nc.tensor.matmul(ps[:, 0:1], gmask[:], zsum[:], start=True, stop=True)
        nc.tensor.matmul(ps[:, 1:2], gmask[:], z2sum[:], start=True, stop=True)
        stats = work.tile([GROUPS_LO, 4], F32, tag="stats")
        mu = stats[:, 0:1]
        e2 = stats[:, 1:2]
        nc.scalar.mul(mu, ps[:, 0:1], 1.0 / (8 * P))
        nc.scalar.mul(e2, ps[:, 1:2], 1.0 / (8 * P))
        var = stats[:, 2:3]
        nc.vector.tensor_mul(var, mu, mu)
        nc.vector.tensor_sub(var, e2, var)
        sd = stats[:, 3:4]
        nc.scalar.activation(sd, var, AF.Sqrt, bias=eps_t[:])
        muinv = work.tile([GROUPS_LO, 2], F32, tag="muinv")
        nc.vector.tensor_copy(muinv[:, 0:1], mu)
        nc.vector.reciprocal(muinv[:, 1:2], sd)
        pbc = psum.tile([128, 2], F32, tag="mm", name="pbc", padded_shape=[128, 512])
        nc.tensor.matmul(pbc[:], bmask[:], muinv[:], start=True, stop=True)
        chan = work.tile([128, 4], F32, tag="chan")
        nc.vector.tensor_copy(chan[:, 0:2], pbc[:])
        mu_c = chan[:, 0:1]
        inv_c = chan[:, 1:2]
        s_c = chan[:, 2:3]
        t_c = chan[:, 3:4]
        nc.vector.tensor_mul(s_c, gg_t[:], inv_c)
        nc.vector.tensor_mul(t_c, mu_c, s_c)
        nc.vector.tensor_sub(t_c, gb2_t[:], t_c)

        R = work.tile([128, P], F32, tag="big4", name="R", bufs=2)
        nc.vector.tensor_scalar(R[:], Z[:], s_c, t_c, op0=ALU.mult, op1=ALU.add)
        nc.vector.tensor_add(R[:], R[:], B0[:])

        # ======== phase D: scatter (in-place into resident tiles) ========
        nc.vector.tensor_add(f2o[:], R[:], f2o[:])
        nc.sync.dma_start(dram["o2"][:], f2o[:])

        o3t = out_stage.tile([128, 256], F32, tag="o3t", bufs=1)
        nc.vector.reduce_max(
            o3t[:],
            R[:].rearrange("p (h dy w dx) -> p h w dy dx", h=16, dy=2, w=16, dx=2),
            axis=AX.XY,
        )
        nc.vector.tensor_add(o3t[:], o3t[:], f3o[:])
        nc.sync.dma_start(dram["o3"][:], o3t[:])

        o4t = out_stage.tile([128, 64], F32, tag="o4t", bufs=1)
        nc.vector.reduce_max(
            o4t[:],
            R[:].rearrange("p (h dy w dx) -> p h w dy dx", h=8, dy=4, w=8, dx=4),
            axis=AX.XY,
        )
        nc.vector.tensor_add(o4t[:], o4t[:], f4o[:])
        nc.sync.dma_start(dram["o4"][:], o4t[:])

        for b in range(2):
            ov = f1o[:, b * 2048 : (b + 1) * 2048].rearrange(
                "p (h r w s) -> p h r w s", h=16, r=2, w=32, s=2
            )
            sv = (
                R[:, b * 512 : (b + 1) * 512]
                .rearrange("p (h w s) -> p h w s", h=16, w=32, s=1)
                .broadcast_to([128, 16, 32, 2])
            )
            for r in range(2):
                nc.vector.tensor_add(ov[:, :, r], ov[:, :, r], sv)
            nc.sync.dma_start(
                dram["o1"][:, b * 2048 : (b + 1) * 2048], f1o[:, b * 2048 : (b + 1) * 2048]
            )
        for b in range(4):
            ov = f0o[:, b * 4096 : (b + 1) * 4096].rearrange(
                "p (h r w s) -> p h r w s", h=8, r=4, w=32, s=4
            )
            sv = (
                R[:, b * 256 : (b + 1) * 256]
                .rearrange("p (h w s) -> p h w s", h=8, w=32, s=1)
                .broadcast_to([128, 8, 32, 4])
            )
            for r in range(4):
                nc.vector.tensor_add(ov[:, :, r], ov[:, :, r], sv)
            nc.sync.dma_start(
                dram["o0"][:, b * 4096 : (b + 1) * 4096], f0o[:, b * 4096 : (b + 1) * 4096]
            )


def _get_program():
    if "nc" not in _CACHE:
        _CACHE["nc"] = build_program()
    return _CACHE["nc"]


def kernel(**inputs):
    f = [np.asarray(inputs[f"f{i}"], np.float32) for i in range(5)]
    n_batch = f[0].shape[0]
    w = {k: np.asarray(inputs[k], np.float32) for k in ("g_w", "th_w", "ph_w", "out_w")}
    b = {k: np.asarray(inputs[k], np.float32) for k in ("g_b", "th_b", "ph_b", "out_b")}
    gn_g = np.asarray(inputs["gn_g"], np.float32)
    gn_b = np.asarray(inputs["gn_b"], np.float32)

    perms = [np.arange(C), np.concatenate([np.arange(128, 256), np.arange(0, 128)])]

    gm = np.zeros((128, GROUPS_LO), np.float32)
    gm[np.arange(128), np.arange(128) // 8] = 1.0
    ident = np.eye(128, dtype=np.float32)

    in_maps = []
    for k in range(N_CORES):
        n, h = k // 2, k % 2
        pm = perms[h]
        m = {}
        for i in range(5):
            fi = f[i][n][pm]
            s = SIZES[i]
            m[f"f{i}o"] = np.ascontiguousarray(fi[:H].reshape(H, s * s))
            m[f"f{i}x"] = np.ascontiguousarray(fi[H:].reshape(H, s * s))
        # permuted, bias-augmented weights
        for name, wk, bk in (
            ("wtA", "th_w", "th_b"),
            ("wpA", "ph_w", "ph_b"),
            ("wgA", "g_w", "g_b"),
        ):
            wp = w[wk][pm][:, pm]
            bp = b[bk][pm]
            m[name] = np.ascontiguousarray(np.concatenate([wp, bp[:, None]], axis=1))
        wo_p = w["out_w"][pm][:, pm]
        m["wol"] = np.ascontiguousarray(wo_p[:H, :].T)
        m["bo"] = np.ascontiguousarray(b["out_b"][pm])
        m["gg"] = np.ascontiguousarray(gn_g[pm][:H])
        m["gb2"] = np.ascontiguousarray(gn_b[pm][:H])
        m["gmask"] = gm
        m["bmask"] = np.ascontiguousarray(gm.T)
        m["ident"] = ident
        in_maps.append(m)

    nc = _get_program()
    kw = {}
    if _CACHE.get("profile"):
        kw["trace"] = True
    res = run_bass_kernel_spmd(nc, in_maps, core_ids=list(range(N_CORES)), **kw)
    _CACHE["last_result"] = res

    outs = []
    for i, s in enumerate(SIZES):
        o = np.empty((n_batch, C, s, s), np.float32)
        for k in range(N_CORES):
            n, h = k // 2, k % 2
            pm = perms[h]
            o[n, pm[:H]] = res.results[k][f"o{i}"].reshape(H, s, s)
        outs.append(o)
    return tuple(outs)


# revision 24
# speedup vs baseline: 1.2394x; 1.1084x over previous
"""Balanced Feature Pyramid (Libra R-CNN BFP) on 8 Trainium2 NeuronCores.

Sharding: core k -> (batch n = k//2, channel-half h = k%2).  Every core
redundantly computes the full 256-channel gathered feature + non-local block
for its batch (cheap), but loads/keeps resident only its own 128-channel half
of the pyramid and writes only that half of the outputs.  To keep the program
identical across cores (SPMD), channels are permuted host-side so that the
"own" half is always channels 0..127 in program space; conv weights are
permuted accordingly (w' = P w P^T), which leaves the math exact.

The non-local block is computed without materializing the 1024x1024 attention
matrix: with augmented weights (bias folded as an extra input channel backed
by a ones-row in bsf~),
    z = Wo_lo @ g_x @ pw^T + bo
      = [Wo_lo Wg~] (bsf~ bsf~^T / N) [Wp~^T Wt~] bsf~ + bo
so everything flows through 257x257 intermediates (S~ = bsf~ bsf~^T/N,
T1 = S~ M2~, T2T = T1^T A~^T, z = T2T^T bsf~) - exact fp32 throughout.

Phases (single kernel launch, no collectives):
  A: load own half resident; stream+pool other half; assemble bsf.
  C: transposed bsf~ via PE transposes, then the chain above, group norm via
     0/1-mask matmuls + activation accumulators, residual.
  D: scatter: out_i = resize(bsf_refined)[own] + f_i[own], in place.
"""
import contextlib

import numpy as np

import concourse.bass as bass
import concourse.tile as tile
from concourse import bacc, mybir
from concourse.bass_utils import run_bass_kernel_spmd

F32 = mybir.dt.float32
AF = mybir.ActivationFunctionType
AX = mybir.AxisListType
ALU = mybir.AluOpType

N_CORES = 8
C = 256
H = 128  # channels per core (own half)
GS = 32
P = GS * GS  # 1024 positions
CA = C + 1  # augmented channels (ones row)
GROUPS_LO = 16
EPS = 1e-5

SIZES = [128, 64, 32, 16, 8]

_CACHE = {}


def build_program(aug=True):
    nc = bacc.Bacc(
        "TRN2",
        target_bir_lowering=False,
        debug=False,
        enable_asserts=False,
        num_devices=N_CORES,
    )

    dram = {}
    for i, s in enumerate(SIZES):
        dram[f"f{i}o"] = nc.dram_tensor(f"f{i}o", [H, s * s], F32, kind="ExternalInput").ap()
        dram[f"f{i}x"] = nc.dram_tensor(f"f{i}x", [H, s * s], F32, kind="ExternalInput").ap()
    ca = CA if aug else C
    dram["wtA"] = nc.dram_tensor("wtA", [C, ca], F32, kind="ExternalInput").ap()
    dram["wpA"] = nc.dram_tensor("wpA", [C, ca], F32, kind="ExternalInput").ap()
    dram["wgA"] = nc.dram_tensor("wgA", [C, ca], F32, kind="ExternalInput").ap()
    dram["wol"] = nc.dram_tensor("wol", [C, H], F32, kind="ExternalInput").ap()
    dram["bo"] = nc.dram_tensor("bo", [C], F32, kind="ExternalInput").ap()
    dram["gg"] = nc.dram_tensor("gg", [H], F32, kind="ExternalInput").ap()
    dram["gb2"] = nc.dram_tensor("gb2", [H], F32, kind="ExternalInput").ap()
    dram["gmask"] = nc.dram_tensor("gmask", [128, GROUPS_LO], F32, kind="ExternalInput").ap()
    dram["bmask"] = nc.dram_tensor("bmask", [GROUPS_LO, 128], F32, kind="ExternalInput").ap()
    dram["ident"] = nc.dram_tensor("ident", [128, 128], F32, kind="ExternalInput").ap()
    for i, s in enumerate(SIZES):
        dram[f"o{i}"] = nc.dram_tensor(f"o{i}", [H, s * s], F32, kind="ExternalOutput").ap()

    with tile.TileContext(nc) as tc:
        _build_body(nc, tc, dram, aug)

    nc.compile()
    return nc


def _build_body(nc, tc, dram, aug=True):
    ca = CA if aug else C
    nm = 3 if aug else 2
    ctx = contextlib.ExitStack()
    with ctx:
        res = ctx.enter_context(tc.tile_pool(name="resident", bufs=1))
        stream = ctx.enter_context(tc.tile_pool(name="stream", bufs=1))
        vpool = ctx.enter_context(tc.tile_pool(name="vpool", bufs=3))
        consts = ctx.enter_context(tc.tile_pool(name="consts", bufs=1))
        work = ctx.enter_context(tc.tile_pool(name="work", bufs=1))
        out_stage = ctx.enter_context(tc.tile_pool(name="ostage", bufs=2))
        psum = ctx.enter_context(tc.tile_pool(name="psum", bufs=3, space="PSUM"))
        psum2 = ctx.enter_context(tc.tile_pool(name="psum2", bufs=2, space="PSUM"))
        psum_z = ctx.enter_context(tc.tile_pool(name="psum_z", bufs=1, space="PSUM"))

        # ======== phase A load issuance ========
        B0 = work.tile([128, P], F32, tag="B0")
        B1 = work.tile([128, P], F32, tag="B1")
        f1o = res.tile([128, 4096], F32, tag="f1o")
        for b in range(2):
            nc.sync.dma_start(
                f1o[:, b * 2048 : (b + 1) * 2048], dram["f1o"][:, b * 2048 : (b + 1) * 2048]
            )
        v1x = stream.tile([128, 4096], F32, tag="v1x")
        for b in range(2):
            nc.sync.dma_start(
                v1x[:, b * 2048 : (b + 1) * 2048], dram["f1x"][:, b * 2048 : (b + 1) * 2048]
            )
        f2o = res.tile([128, P], F32, tag="f2o")
        nc.sync.dma_start(f2o[:], dram["f2o"][:])
        f2x = stream.tile([128, P], F32, tag="f2x")
        nc.sync.dma_start(f2x[:], dram["f2x"][:])
        f3o = res.tile([128, 256], F32, tag="f3o")
        nc.sync.dma_start(f3o[:], dram["f3o"][:])
        f3x = stream.tile([128, 256], F32, tag="f3x")
        nc.sync.dma_start(f3x[:], dram["f3x"][:])
        f4o = res.tile([128, 64], F32, tag="f4o")
        nc.sync.dma_start(f4o[:], dram["f4o"][:])
        f4x = stream.tile([128, 64], F32, tag="f4x")
        nc.sync.dma_start(f4x[:], dram["f4x"][:])
        f0o = res.tile([128, 16384], F32, tag="f0o")
        for b in range(4):
            nc.sync.dma_start(
                f0o[:, b * 4096 : (b + 1) * 4096], dram["f0o"][:, b * 4096 : (b + 1) * 4096]
            )
        v0xs = []
        for b in range(8):
            v0x = vpool.tile([128, 2048], F32, tag="v0x", name=f"v0x{b}", bufs=6)
            nc.sync.dma_start(v0x[:], dram["f0x"][:, b * 2048 : (b + 1) * 2048])
            v0xs.append(v0x)

        # f1 pools first (DVE) - they gate Bs
        p1o = vpool.tile([128, P], F32, tag="scratch", name="p1o", bufs=2)
        v1o = f1o[:].rearrange("p (h dy w dx) -> p h w dy dx", h=32, dy=2, w=32, dx=2)
        nc.vector.reduce_max(p1o[:], v1o, axis=AX.XY)
        p1x = vpool.tile([128, P], F32, tag="scratch", name="p1x", bufs=2)
        v1xv = v1x[:].rearrange("p (h dy w dx) -> p h w dy dx", h=32, dy=2, w=32, dx=2)
        nc.vector.reduce_max(p1x[:], v1xv, axis=AX.XY)

        # ======== const loads ========
        wA = {}
        for wname in ("wtA", "wpA", "wgA"):
            for k in range(2):
                t = consts.tile([128, ca], F32, tag=f"{wname}{k}", name=f"{wname}{k}")
                nc.sync.dma_start(t[:], dram[wname][k * 128 : (k + 1) * 128, :])
                wA[f"{wname}{k}"] = t
        wol = []
        for k in range(2):
            t = consts.tile([128, H], F32, tag=f"wol{k}", name=f"wol{k}")
            nc.sync.dma_start(t[:], dram["wol"][k * 128 : (k + 1) * 128, :])
            wol.append(t)
        ident = consts.tile([128, 128], F32, tag="ident")
        nc.sync.dma_start(ident[:], dram["ident"][:])
        bo0 = consts.tile([128, 1], F32, tag="bo0")
        nc.sync.dma_start(bo0[:], dram["bo"].rearrange("(c o) -> c o", o=1)[0:128])
        gg_t = consts.tile([128, 1], F32, tag="gg")
        nc.sync.dma_start(gg_t[:], dram["gg"].rearrange("(c o) -> c o", o=1))
        gb2_t = consts.tile([128, 1], F32, tag="gb2")
        nc.sync.dma_start(gb2_t[:], dram["gb2"].rearrange("(c o) -> c o", o=1))
        gmask = consts.tile([128, GROUPS_LO], F32, tag="gmask")
        nc.sync.dma_start(gmask[:], dram["gmask"][:])
        bmask = consts.tile([GROUPS_LO, 128], F32, tag="bmask")
        nc.sync.dma_start(bmask[:], dram["bmask"][:])
        eps_t = consts.tile([GROUPS_LO, 1], F32, tag="eps_t")
        nc.gpsimd.memset(eps_t[:], EPS)
        ones_row = None
        if aug:
            ones_row = consts.tile([1, P], F32, tag="ones_row")
            nc.gpsimd.memset(ones_row[:], 1.0)

        # ---- M2~ = WpA^T @ WtA  [257, 257] in chunks (128,128,1) ----
        M2 = []
        for m in range(2):
            pm = psum.tile([128, ca], F32, tag="mm", name=f"pm2_{m}", padded_shape=[128, 512])
            for k in range(2):
                nc.tensor.matmul(
                    pm[:],
                    wA[f"wpA{k}"][:, m * 128 : (m + 1) * 128],
                    wA[f"wtA{k}"][:],
                    start=(k == 0),
                    stop=(k == 1),
                )
            t = consts.tile([128, ca], F32, tag=f"M2_{m}", name=f"M2_{m}")
            nc.scalar.copy(t[:], pm[:])
            M2.append(t)
        if aug:
            pmr = psum.tile([1, ca], F32, tag="mm", name="pm2r", padded_shape=[128, 512])
            for k in range(2):
                nc.tensor.matmul(
                    pmr[:], wA[f"wpA{k}"][:, 256:257], wA[f"wtA{k}"][:],
                    start=(k == 0), stop=(k == 1),
                )
            M2r = consts.tile([1, ca], F32, tag="M2r")
            nc.scalar.copy(M2r[:], pmr[:])
            M2.append(M2r)

        # ---- AT~ = (Wo_lo @ WgA)^T  [257, 128] in chunks ----
        AT = []
        for m in range(2):
            pa = psum.tile([128, H], F32, tag="mm", name=f"pat{m}", padded_shape=[128, 512])
            for k in range(2):
                nc.tensor.matmul(
                    pa[:],
                    wA[f"wgA{k}"][:, m * 128 : (m + 1) * 128],
                    wol[k][:],
                    start=(k == 0),
                    stop=(k == 1),
                )
            t = consts.tile([128, H], F32, tag=f"AT{m}", name=f"AT{m}")
            nc.scalar.copy(t[:], pa[:])
            AT.append(t)
        if aug:
            par = psum.tile([1, H], F32, tag="mm", name="patr", padded_shape=[128, 512])
            for k in range(2):
                nc.tensor.matmul(
                    par[:], wA[f"wgA{k}"][:, 256:257], wol[k][:],
                    start=(k == 0), stop=(k == 1),
                )
            ATr = consts.tile([1, H], F32, tag="ATr")
            nc.scalar.copy(ATr[:], par[:])
            AT.append(ATr)

        # ======== phase A: bsf assembly ========
        def ups_add(B, small, hs, f, engine):
            for r in range(f):
                ov = B.rearrange("p (h r w s) -> p h r w s", h=hs, r=f, w=hs, s=f)[:, :, r]
                sv = small.rearrange("p (h w s) -> p h w s", h=hs, w=hs, s=1).broadcast_to(
                    [128, hs, hs, f]
                )
                engine.tensor_add(ov, ov, sv)

        # small-level accumulators built early (only small loads + f1 pools needed)
        Bs0 = work.tile([128, P], F32, tag="big4", name="Bs0", bufs=2)
        Bs1 = work.tile([128, P], F32, tag="big4", name="Bs1", bufs=2)
        nc.vector.tensor_add(Bs0[:], p1o[:], f2o[:])
        nc.vector.tensor_add(Bs1[:], p1x[:], f2x[:])
        ups_add(Bs0[:], f3o[:], 16, 2, nc.vector)
        ups_add(Bs1[:], f3x[:], 16, 2, nc.vector)
        ups_add(Bs0[:], f4o[:], 8, 4, nc.vector)
        ups_add(Bs1[:], f4x[:], 8, 4, nc.vector)
        # pre-scale the small-level accumulators (early, off the critical path)
        nc.scalar.mul(Bs0[:], Bs0[:], 0.2)
        nc.scalar.mul(Bs1[:], Bs1[:], 0.2)
        # f0 own pooling + B0 finalize per chunk
        for b in range(4):
            view = f0o[:, b * 4096 : (b + 1) * 4096].rearrange(
                "p (h dy w dx) -> p h w dy dx", h=8, dy=4, w=32, dx=4
            )
            nc.vector.reduce_max(B0[:, b * 256 : (b + 1) * 256], view, axis=AX.XY)
            sl = slice(b * 256, (b + 1) * 256)
            nc.vector.scalar_tensor_tensor(
                B0[:, sl], B0[:, sl], 0.2, Bs0[:, sl], op0=ALU.mult, op1=ALU.add
            )
        B = [B0, B1]

        # interleaved stream: pool f0x chunk q -> finalize B1 block q ->
        # transpose block q -> S~ accumulation step q
        bsfT = []
        pS = []
        for m in range(nm):
            rows = 128 if m < 2 else 1
            pS.append(
                psum.tile([rows, ca], F32, tag="mm", name=f"pS{m}", padded_shape=[128, 512])
            )
        for q in range(8):
            sl = slice(q * 128, (q + 1) * 128)
            view = v0xs[q][:].rearrange(
                "p (h dy w dx) -> p h w dy dx", h=4, dy=4, w=32, dx=4
            )
            nc.vector.reduce_max(B1[:, sl], view, axis=AX.XY)
            nc.vector.scalar_tensor_tensor(
                B1[:, sl], B1[:, sl], 0.2, Bs1[:, sl], op0=ALU.mult, op1=ALU.add
            )
            pt = psum2.tile([128, 256], F32, tag="ptr", name=f"ptr{q}")
            for m in range(2):
                nc.tensor.transpose(
                    pt[:, m * 128 : (m + 1) * 128],
                    B[m][:, q * 128 : (q + 1) * 128],
                    ident[:],
                )
            t = work.tile([128, ca], F32, tag="bsfT", name=f"bsfT{q}", bufs=3)
            nc.scalar.copy(t[:, 0:256], pt[:])
            if aug:
                nc.gpsimd.memset(t[:, 256:257], 1.0)
            bsfT.append(t)
            for m in range(nm):
                lhs = (
                    t[:, m * 128 : (m + 1) * 128] if m < 2 else t[:, 256:257]
                )
                nc.tensor.matmul(pS[m][:], lhs, t[:], start=(q == 0), stop=(q == 7))

        S = []
        for m in range(nm):
            rows = 128 if m < 2 else 1
            t = work.tile([rows, ca], F32, tag=f"S{m}", name=f"S{m}")
            nc.scalar.mul(t[:], pS[m][:], 1.0 / P)
            S.append(t)

        # T1 = S~ @ M2~ (S symmetric -> S rows used as lhsT)
        T1 = []
        for m in range(nm):
            rows = 128 if m < 2 else 1
            pT = psum.tile([rows, ca], F32, tag="mm", name=f"pT1{m}", padded_shape=[128, 512])
            for k in range(nm):
                lhs = S[k][:, m * 128 : (m + 1) * 128] if m < 2 else S[k][:, 256:257]
                nc.tensor.matmul(pT[:], lhs, M2[k][:], start=(k == 0), stop=(k == nm - 1))
            t = work.tile([rows, ca], F32, tag=f"T1_{m}", name=f"T1_{m}")
            nc.scalar.copy(t[:], pT[:])
            T1.append(t)

        # T2T = T1^T @ A~^T : [257, 128] chunks
        T2T = []
        for m in range(nm):
            rows = 128 if m < 2 else 1
            pZ = psum.tile([rows, H], F32, tag="mm", name=f"pT2{m}", padded_shape=[128, 512])
            for k in range(nm):
                lhs = T1[k][:, m * 128 : (m + 1) * 128] if m < 2 else T1[k][:, 256:257]
                nc.tensor.matmul(pZ[:], lhs, AT[k][:], start=(k == 0), stop=(k == nm - 1))
            t = work.tile([rows, H], F32, tag=f"T2T{m}", name=f"T2T{m}")
            nc.scalar.copy(t[:], pZ[:])
            T2T.append(t)

        # z = T2T^T @ bsf~  [128, 1024]
        pz = psum_z.tile([128, P], F32, tag="pz")
        for i in range(2):
            sl = slice(i * 512, (i + 1) * 512)
            for k in range(nm):
                rhs = B[k][:, sl] if k < 2 else ones_row[:, sl]
                nc.tensor.matmul(pz[:, sl], T2T[k][:], rhs, start=(k == 0), stop=(k == nm - 1))

        # GN stats via activation accumulators + mask matmuls
        Z = work.tile([128, P], F32, tag="big4", name="Z", bufs=2)
        zsum = work.tile([128, 1], F32, tag="zsum")
        nc.scalar.activation(Z[:], pz[:], AF.Identity, bias=bo0[:], accum_out=zsum[:])
        Z2 = vpool.tile([128, P], F32, tag="scratch", name="Z2", bufs=2)
        z2sum = work.tile([128, 1], F32, tag="z2sum")
        nc.scalar.activation(Z2[:], pz[:], AF.Square, bias=bo0[:], accum_out=z2sum[:])

        ps = psum.tile([GROUPS_LO, 2], F32, tag="mm", name="ps", padded_shape=[128, 512])
        nc.tensor.matmul(ps[:, 0:1], gmask[:], zsum[:], start=True, stop=True)
        nc.tensor.matmul(ps[:, 1:2], gmask[:], z2sum[:], start=True, stop=True)
        stats = work.tile([GROUPS_LO, 4], F32, tag="stats")
        mu = stats[:, 0:1]
        e2 = stats[:, 1:2]
        nc.scalar.mul(mu, ps[:, 0:1], 1.0 / (8 * P))
        nc.scalar.mul(e2, ps[:, 1:2], 1.0 / (8 * P))
        var = stats[:, 2:3]
        nc.vector.tensor_mul(var, mu, mu)
        nc.vector.tensor_sub(var, e2, var)
        sd = stats[:, 3:4]
        nc.scalar.activation(sd, var, AF.Sqrt, bias=eps_t[:])
        muinv = work.tile([GROUPS_LO, 2], F32, tag="muinv")
        nc.vector.tensor_copy(muinv[:, 0:1], mu)
        nc.vector.reciprocal(muinv[:, 1:2], sd)
        pbc = psum.tile([128, 2], F32, tag="mm", name="pbc", padded_shape=[128, 512])
        nc.tensor.matmul(pbc[:], bmask[:], muinv[:], start=True, stop=True)
        chan = work.tile([128, 4], F32, tag="chan")
        nc.vector.tensor_copy(chan[:, 0:2], pbc[:])
        mu_c = chan[:, 0:1]
        inv_c = chan[:, 1:2]
        s_c = chan[:, 2:3]
        t_c = chan[:, 3:4]
        nc.vector.tensor_mul(s_c, gg_t[:], inv_c)
        nc.vector.tensor_mul(t_c, mu_c, s_c)
        nc.vector.tensor_sub(t_c, gb2_t[:], t_c)

        R = work.tile([128, P], F32, tag="big4", name="R", bufs=2)
        nc.vector.tensor_scalar(R[:], Z[:], s_c, t_c, op0=ALU.mult, op1=ALU.add)
        nc.vector.tensor_add(R[:], R[:], B0[:])

        # ======== phase D: scatter (in-place into resident tiles) ========
        nc.vector.tensor_add(f2o[:], R[:], f2o[:])
        nc.sync.dma_start(dram["o2"][:], f2o[:])

        o3t = out_stage.tile([128, 256], F32, tag="o3t", bufs=1)
        nc.vector.reduce_max(
            o3t[:],
            R[:].rearrange("p (h dy w dx) -> p h w dy dx", h=16, dy=2, w=16, dx=2),
            axis=AX.XY,
        )
        nc.vector.tensor_add(o3t[:], o3t[:], f3o[:])
        nc.sync.dma_start(dram["o3"][:], o3t[:])

        o4t = out_stage.tile([128, 64], F32, tag="o4t", bufs=1)
        nc.vector.reduce_max(
            o4t[:],
            R[:].rearrange("p (h dy w dx) -> p h w dy dx", h=8, dy=4, w=8, dx=4),
            axis=AX.XY,
        )
        nc.vector.tensor_add(o4t[:], o4t[:], f4o[:])
        nc.sync.dma_start(dram["o4"][:], o4t[:])

        for b in range(2):
            ov = f1o[:, b * 2048 : (b + 1) * 2048].rearrange(
                "p (h r w s) -> p h r w s", h=16, r=2, w=32, s=2
            )
            sv = (
                R[:, b * 512 : (b + 1) * 512]
                .rearrange("p (h w s) -> p h w s", h=16, w=32, s=1)
                .broadcast_to([128, 16, 32, 2])
            )
            for r in range(2):
                nc.vector.tensor_add(ov[:, :, r], ov[:, :, r], sv)
            nc.sync.dma_start(
                dram["o1"][:, b * 2048 : (b + 1) * 2048], f1o[:, b * 2048 : (b + 1) * 2048]
            )
        for b in range(4):
            ov = f0o[:, b * 4096 : (b + 1) * 4096].rearrange(
                "p (h r w s) -> p h r w s", h=8, r=4, w=32, s=4
            )
            sv = (
                R[:, b * 256 : (b + 1) * 256]
                .rearrange("p (h w s) -> p h w s", h=8, w=32, s=1)
                .broadcast_to([128, 8, 32, 4])
            )
            for r in range(4):
                nc.vector.tensor_add(ov[:, :, r], ov[:, :, r], sv)
            nc.sync.dma_start(
                dram["o0"][:, b * 4096 : (b + 1) * 4096], f0o[:, b * 4096 : (b + 1) * 4096]
            )


def _get_program(aug):
    key = f"nc_aug{int(aug)}"
    if key not in _CACHE:
        _CACHE[key] = build_program(aug)
    return _CACHE[key]


def kernel(**inputs):
    f = [np.asarray(inputs[f"f{i}"], np.float32) for i in range(5)]
    n_batch = f[0].shape[0]
    w = {k: np.asarray(inputs[k], np.float32) for k in ("g_w", "th_w", "ph_w", "out_w")}
    b = {k: np.asarray(inputs[k], np.float32) for k in ("g_b", "th_b", "ph_b", "out_b")}
    gn_g = np.asarray(inputs["gn_g"], np.float32)
    gn_b = np.asarray(inputs["gn_b"], np.float32)

    perms = [np.arange(C), np.concatenate([np.arange(128, 256), np.arange(0, 128)])]

    aug = any(
        np.any(b[k]) for k in ("th_b", "ph_b", "g_b")
    )

    gm = np.zeros((128, GROUPS_LO), np.float32)
    gm[np.arange(128), np.arange(128) // 8] = 1.0
    ident = np.eye(128, dtype=np.float32)

    in_maps = []
    for k in range(N_CORES):
        n, h = k // 2, k % 2
        pm = perms[h]
        m = {}
        for i in range(5):
            fi = f[i][n][pm]
            s = SIZES[i]
            m[f"f{i}o"] = np.ascontiguousarray(fi[:H].reshape(H, s * s))
            m[f"f{i}x"] = np.ascontiguousarray(fi[H:].reshape(H, s * s))
        # permuted, bias-augmented weights
        for name, wk, bk in (
            ("wtA", "th_w", "th_b"),
            ("wpA", "ph_w", "ph_b"),
            ("wgA", "g_w", "g_b"),
        ):
            wp = w[wk][pm][:, pm]
            if aug:
                bp = b[bk][pm]
                m[name] = np.ascontiguousarray(
                    np.concatenate([wp, bp[:, None]], axis=1)
                )
            else:
                m[name] = np.ascontiguousarray(wp)
        wo_p = w["out_w"][pm][:, pm]
        m["wol"] = np.ascontiguousarray(wo_p[:H, :].T)
        m["bo"] = np.ascontiguousarray(b["out_b"][pm])
        m["gg"] = np.ascontiguousarray(gn_g[pm][:H])
        m["gb2"] = np.ascontiguousarray(gn_b[pm][:H])
        m["gmask"] = gm
        m["bmask"] = np.ascontiguousarray(gm.T)
        m["ident"] = ident
        in_maps.append(m)

    nc = _get_program(aug)
    kw = {}
    if _CACHE.get("profile"):
        kw["trace"] = True
    res = run_bass_kernel_spmd(nc, in_maps, core_ids=list(range(N_CORES)), **kw)
    _CACHE["last_result"] = res

    outs = []
    for i, s in enumerate(SIZES):
        o = np.empty((n_batch, C, s, s), np.float32)
        for k in range(N_CORES):
            n, h = k // 2, k % 2
            pm = perms[h]
            o[n, pm[:H]] = res.results[k][f"o{i}"].reshape(H, s, s)
        outs.append(o)
    return tuple(outs)
